# revision 1
# baseline (speedup 1.0000x reference)
"""Differential multi-headed attention on 8 Trainium2 NeuronCores.

Sharding: core c = (batch b = c // 2, head-group g = c % 2).  Each core
computes 4 of the 8 differential heads for one batch element, including
the Q/K/V projections restricted to its 512 output dims, the differential
attention, and a partial output projection.  The host sums the two
partial outputs per batch (the "all-reduce"), un-permutes rows, and adds
the output bias.

Device layout notes:
 - All matmul contractions run over the SBUF partition axis, so the host
   ships x^T (and column-permuted x_v^T) instead of x.  The value-row
   permutation makes the reference's row-major [B,dh,N,dk]->[B,2dh,N,dk/2]
   reshape land on plain slices on device.
 - Scores are computed transposed (keys on partitions); softmax
   normalization is applied after the attention*V matmul as a per-column
   scale.  Sums-of-exp use a DVE/GPSIMD fold tree plus a ones-matmul
   partition reduction, broadcast back via a DRAM bounce, and a fast
   approximate reciprocal.
 - Matmul operands are bf16 (fp32 PSUM accumulation); fp32 elsewhere.
"""

import math
from contextlib import ExitStack

import ml_dtypes
import numpy as np

import concourse.bass as bass
import concourse.mybir as mybir
from concourse import bacc
import concourse.tile as tile
from concourse.bass_utils import run_bass_kernel_spmd

F32 = mybir.dt.float32
BF16 = mybir.dt.bfloat16
AF = mybir.ActivationFunctionType
ALU = mybir.AluOpType

N = 1024          # sequence length
D = 1024          # model dim
HG = 512          # head-group dims per core (4 heads x 128)
NHEAD = 4         # local heads per core
SCALE = 1.0 / math.sqrt(64.0)   # 1/sqrt(dk/2)
LAMBDA_INIT = 0.8

_BUILT = None     # cached Bass module -- building + compiling is expensive
LAST_RESULT = None  # BassKernelResults from the most recent run (for test.py)


def _build():
    nc = bacc.Bacc()

    # ---- DRAM I/O (per core) ----
    xqT = nc.dram_tensor("xqT", [D, N], BF16, kind="ExternalInput")
    xkT = nc.dram_tensor("xkT", [D, N], BF16, kind="ExternalInput")
    xvT = nc.dram_tensor("xvT", [D, N], BF16, kind="ExternalInput")  # col-permuted
    wqT = nc.dram_tensor("wqT", [D, HG], BF16, kind="ExternalInput")
    wkT = nc.dram_tensor("wkT", [D, HG], BF16, kind="ExternalInput")
    wvT = nc.dram_tensor("wvT", [D, HG], BF16, kind="ExternalInput")
    woT = nc.dram_tensor("woT", [HG, D], BF16, kind="ExternalInput")
    bq = nc.dram_tensor("bq", [HG], F32, kind="ExternalInput")
    bk = nc.dram_tensor("bk", [HG], F32, kind="ExternalInput")
    bv = nc.dram_tensor("bv", [HG], F32, kind="ExternalInput")
    lamneg = nc.dram_tensor("lamneg", [1, 1], F32, kind="ExternalInput")
    out = nc.dram_tensor("out", [N, D], F32, kind="ExternalOutput")

    with tile.TileContext(nc) as tc, ExitStack() as ctx:
        const = ctx.enter_context(tc.tile_pool(name="const", bufs=1))
        ones_bf = const.tile([128, 1], BF16, name="ones_bf")
        nc.vector.memset(ones_bf[:], 1.0)
        lamneg_bc = const.tile([128, 1], F32, name="lamneg_bc")
        nc.sync.dma_start(out=lamneg_bc[:], in_=lamneg[0, :].partition_broadcast(128))
        bq_sb = const.tile([128, 4], F32, name="bq_sb")
        nc.sync.dma_start(out=bq_sb[:], in_=bq.rearrange("(t p) -> p t", p=128))
        bk_sb = const.tile([128, 4], F32, name="bk_sb")
        nc.sync.dma_start(out=bk_sb[:], in_=bk.rearrange("(t p) -> p t", p=128))
        bv_sb = const.tile([128, HG], F32, name="bv_sb")
        nc.sync.dma_start(out=bv_sb[:], in_=bv[None, :].to_broadcast([128, HG]))

        # Persistent activations
        qkv_pool = ctx.enter_context(tc.tile_pool(name="qkv", bufs=1))
        qt = [qkv_pool.tile([128, N], BF16, name=f"qt{t}") for t in range(4)]
        # qsw[j]: partitions 0:64 = qt[j][64:128] (u=1), 64:128 = qt[j][0:64]
        # (u=0) -- lets every (w, u) scores matmul read lhsT and rhs from the
        # same base partition.
        qsw = [qkv_pool.tile([128, N], BF16, name=f"qsw{t}") for t in range(4)]
        kt = [qkv_pool.tile([128, N], BF16, name=f"kt{t}") for t in range(4)]
        vv = [qkv_pool.tile([128, HG], BF16, name=f"vv{m}") for m in range(8)]
        oh = [qkv_pool.tile([128, N], BF16, name=f"oh{j}") for j in range(NHEAD)]
        wo_sb = [qkv_pool.tile([128, N], BF16, name=f"wo{t}") for t in range(4)]
        for t in range(4):
            nc.sync.dma_start(out=wo_sb[t][:], in_=woT[t * 128:(t + 1) * 128, :])

        # All pools stay open for the whole kernel: releasing a pool and
        # re-using its SBUF region concentrates WAR waits from every DMA
        # queue onto the first instruction of the next phase, which
        # overflows the per-instruction sync-wait limit in codegen.
        xpool = ctx.enter_context(tc.tile_pool(name="xw", bufs=8))
        wpool = ctx.enter_context(tc.tile_pool(name="wp", bufs=8))
        pa = ctx.enter_context(tc.tile_pool(name="psA", bufs=2, space="PSUM"))
        pb = ctx.enter_context(tc.tile_pool(name="psB", bufs=2, space="PSUM"))
        epool = ctx.enter_context(tc.tile_pool(name="epool", bufs=24))
        fpool = ctx.enter_context(tc.tile_pool(name="fold", bufs=6))
        coefp = ctx.enter_context(tc.tile_pool(name="coefp", bufs=4))
        bcastp = ctx.enter_context(tc.tile_pool(name="bcastp", bufs=4))
        drb = ctx.enter_context(tc.tile_pool(name="drb", bufs=4, space="DRAM"))
        combp = ctx.enter_context(tc.tile_pool(name="combp", bufs=2))
        ostg = ctx.enter_context(tc.tile_pool(name="ostg", bufs=4))
        osb = ctx.enter_context(tc.tile_pool(name="osb", bufs=4))

        # ---- Phase 1: projections ----
        if True:
            def proj_qk(x_dram, w_dram, dst, bias_sb):
                xs = []
                ws = []
                for d in range(8):
                    xst = xpool.tile([128, N], BF16, name=f"xs{d}", tag="x")
                    nc.sync.dma_start(out=xst[:], in_=x_dram[d * 128:(d + 1) * 128, :])
                    xs.append(xst)
                    wst = wpool.tile([128, HG], BF16, name=f"ws{d}", tag="w")
                    nc.sync.dma_start(out=wst[:], in_=w_dram[d * 128:(d + 1) * 128, :])
                    ws.append(wst)
                for t in range(4):
                    for half in range(2):
                        pp = pa if (t * 2 + half) % 2 == 0 else pb
                        ps = pp.tile([128, 512], F32, name="pqk", tag="ps")
                        for d in range(8):
                            nc.tensor.matmul(
                                ps[:],
                                ws[d][:, t * 128:(t + 1) * 128],
                                xs[d][:, half * 512:(half + 1) * 512],
                                start=(d == 0), stop=(d == 7),
                            )
                        nc.scalar.activation(
                            dst[t][:, half * 512:(half + 1) * 512], ps[:],
                            AF.Identity, bias=bias_sb[:, t:t + 1],
                        )

            proj_qk(xqT, wqT, qt, bq_sb)
            proj_qk(xkT, wkT, kt, bk_sb)
            for j in range(NHEAD):
                nc.sync.dma_start(out=qsw[j][0:64, :], in_=qt[j][64:128, :])
                nc.sync.dma_start(out=qsw[j][64:128, :], in_=qt[j][0:64, :])

            # V projection: out rows = sequence (permuted), cols = head dims
            xs = []
            ws = []
            for d in range(8):
                xst = xpool.tile([128, N], BF16, name=f"xv{d}", tag="x")
                nc.sync.dma_start(out=xst[:], in_=xvT[d * 128:(d + 1) * 128, :])
                xs.append(xst)
                wst = wpool.tile([128, HG], BF16, name=f"wv{d}", tag="w")
                nc.sync.dma_start(out=wst[:], in_=wvT[d * 128:(d + 1) * 128, :])
                ws.append(wst)
            for m in range(8):
                pp = pa if m % 2 == 0 else pb
                ps = pp.tile([128, 512], F32, name="pv", tag="ps")
                for d in range(8):
                    nc.tensor.matmul(
                        ps[:],
                        xs[d][:, m * 128:(m + 1) * 128],
                        ws[d][:],
                        start=(d == 0), stop=(d == 7),
                    )
                nc.vector.tensor_tensor(
                    out=vv[m][:], in0=ps[:], in1=bv_sb[:], op=ALU.add,
                )

        # ---- Phase 2: attention per head ----
        if True:
            for j in range(NHEAD):
                o_sb = {}
                sums = {}
                for half, lo in ((0, 0), (1, 512)):
                    # scores + exp + AV, woven per chunk. w alternates so
                    # consecutive K=64 matmuls land on different PE row
                    # groups and run concurrently; the AV matmuls fill PE
                    # gaps while ACT drains the next score chunk.
                    etiles = [None] * 8
                    ops = pb.tile([128, N], F32, name="ops", tag="ps")
                    for mc in range(4):
                        for w in range(2):
                            mi = w * 4 + mc
                            sp = pa.tile([128, N], F32, name="sp", tag="ps")
                            et = epool.tile([128, N], BF16, name="et", tag="et")
                            for u in range(2):
                                qsrc = qt[j] if u == w else qsw[j]
                                nc.tensor.matmul(
                                    sp[:, u * 512:(u + 1) * 512],
                                    kt[j][w * 64:(w + 1) * 64,
                                          lo + mc * 128:lo + (mc + 1) * 128],
                                    qsrc[w * 64:(w + 1) * 64, lo:lo + 512],
                                    start=True, stop=True,
                                )
                            nc.scalar.activation(et[:], sp[:], AF.Exp, scale=SCALE)
                            etiles[mi] = et
                            for u in range(2):
                                nc.tensor.matmul(
                                    ops[:, u * 512:(u + 1) * 512],
                                    vv[mi][:, j * 128:(j + 1) * 128],
                                    et[:, u * 512:(u + 1) * 512],
                                    start=(mc == 0 and w == 0),
                                    stop=(mc == 3 and w == 1),
                                )

                    # free the PSUM slot immediately so the next AV can start
                    ot = osb.tile([128, N], F32, name="ot", tag="ob")
                    nc.vector.tensor_copy(out=ot[:], in_=ops[:])
                    o_sb[half] = ot

                    # sum over keys: DVE/GPSIMD fold tree + partition reduce
                    a0 = fpool.tile([128, N], BF16, name="a0", tag="f")
                    a1 = fpool.tile([128, N], BF16, name="a1", tag="f")
                    a2 = fpool.tile([128, N], BF16, name="a2", tag="f")
                    a3 = fpool.tile([128, N], BF16, name="a3", tag="f")
                    nc.vector.tensor_tensor(out=a0[:], in0=etiles[0][:], in1=etiles[1][:], op=ALU.add)
                    nc.vector.tensor_tensor(out=a1[:], in0=etiles[2][:], in1=etiles[3][:], op=ALU.add)
                    nc.gpsimd.tensor_tensor(out=a2[:], in0=etiles[4][:], in1=etiles[5][:], op=ALU.add)
                    nc.gpsimd.tensor_tensor(out=a3[:], in0=etiles[6][:], in1=etiles[7][:], op=ALU.add)
                    b0 = fpool.tile([128, N], BF16, name="b0", tag="f")
                    b1 = fpool.tile([128, N], BF16, name="b1", tag="f")
                    ff = fpool.tile([128, N], BF16, name="ff", tag="f")
                    nc.vector.tensor_tensor(out=b0[:], in0=a0[:], in1=a1[:], op=ALU.add)
                    nc.gpsimd.tensor_tensor(out=b1[:], in0=a2[:], in1=a3[:], op=ALU.add)
                    nc.vector.tensor_tensor(out=ff[:], in0=b0[:], in1=b1[:], op=ALU.add)
                    # partition-sum via a cheap ones-matmul, broadcast the
                    # raw sums through DRAM, then fast reciprocal in place
                    sps = pa.tile([1, N], F32, name="sps", tag="ps")
                    for u in range(2):
                        nc.tensor.matmul(
                            sps[:, u * 512:(u + 1) * 512],
                            ones_bf[:],
                            ff[:, u * 512:(u + 1) * 512],
                            start=True, stop=True,
                        )
                    cfs = coefp.tile([1, N], F32, name="cfs", tag="coef")
                    nc.scalar.copy(cfs[:], sps[:])
                    drs = drb.tile([1, N], F32, name="drs", tag="dr")
                    nc.sync.dma_start(out=drs[:], in_=cfs[:])
                    bcx = bcastp.tile([128, N], F32, name="bcx", tag="bc")
                    nc.sync.dma_start(out=bcx[:], in_=drs[0, :].partition_broadcast(128))
                    nc.vector.reciprocal_approx_fast(out=bcx[:], in_=bcx[:])
                    sums[half] = bcx

                abc = sums[0]
                bbc = sums[1]
                t1 = combp.tile([128, N], F32, name="t1", tag="cb")
                t2 = combp.tile([128, N], F32, name="t2", tag="cb")
                nc.vector.tensor_tensor(out=t1[:], in0=o_sb[0][:], in1=abc[:], op=ALU.mult)
                nc.vector.scalar_tensor_tensor(
                    out=t2[:], in0=o_sb[1][:], scalar=lamneg_bc[:], in1=bbc[:],
                    op0=ALU.mult, op1=ALU.mult,
                )
                nc.vector.tensor_tensor(out=oh[j][:], in0=t1[:], in1=t2[:], op=ALU.add)

        # ---- Phase 3: output projection (partial; host sums core pairs) ----
        if True:
            for nci in range(8):
                for half in range(2):
                    pp = pa if (nci * 2 + half) % 2 == 0 else pb
                    ps = pp.tile([128, 512], F32, name="po", tag="ps")
                    for j in range(NHEAD):
                        nc.tensor.matmul(
                            ps[:],
                            oh[j][:, nci * 128:(nci + 1) * 128],
                            wo_sb[j][:, half * 512:(half + 1) * 512],
                            start=(j == 0), stop=(j == NHEAD - 1),
                        )
                    stg = ostg.tile([128, 512], F32, name="stg", tag="og")
                    nc.vector.tensor_copy(out=stg[:], in_=ps[:])
                    nc.sync.dma_start(
                        out=out[nci * 128:(nci + 1) * 128, half * 512:(half + 1) * 512],
                        in_=stg[:],
                    )

    if not nc.is_finalized():
        nc.finalize()
    return nc


def _get_built():
    global _BUILT
    if _BUILT is None:
        _BUILT = _build()
    return _BUILT


def kernel(**inputs):
    inp = {k: np.asarray(v) for k, v in inputs.items()}
    q_, k_, v_ = inp["query"], inp["key"], inp["value"]
    Wq, Wk, Wv, Wo = inp["Wq"], inp["Wk"], inp["Wv"], inp["Wo"]
    bq_, bk_, bv_, bo_ = inp["bq"], inp["bk"], inp["bv"], inp["bo"]
    B = q_.shape[0]

    lam = (np.exp(np.sum(inp["lambda_q1"].astype(np.float64) * inp["lambda_k1"].astype(np.float64)))
           - np.exp(np.sum(inp["lambda_q2"].astype(np.float64) * inp["lambda_k2"].astype(np.float64)))
           + LAMBDA_INIT)

    # value-row permutation: xv'[w*512 + m] = xv[2m + w]
    permv = np.arange(N).reshape(512, 2).T.reshape(-1)  # index i'=w*512+m -> 2m+w

    in_maps = []
    for c in range(8):
        b, g = c // 2, c % 2
        sl = slice(g * HG, (g + 1) * HG)
        bf = ml_dtypes.bfloat16
        in_maps.append({
            "xqT": np.ascontiguousarray(q_[b].T).astype(bf),
            "xkT": np.ascontiguousarray(k_[b].T).astype(bf),
            "xvT": np.ascontiguousarray(v_[b][permv].T).astype(bf),
            "wqT": np.ascontiguousarray(Wq[sl, :].T).astype(bf),
            "wkT": np.ascontiguousarray(Wk[sl, :].T).astype(bf),
            "wvT": np.ascontiguousarray(Wv[sl, :].T).astype(bf),
            "woT": np.ascontiguousarray(Wo[:, sl].T).astype(bf),
            "bq": np.ascontiguousarray(bq_[sl]),
            "bk": np.ascontiguousarray(bk_[sl]),
            "bv": np.ascontiguousarray(bv_[sl]),
            "lamneg": np.array([[-lam]], dtype=np.float32),
        })

    nc = _get_built()
    res = run_bass_kernel_spmd(nc, in_maps, core_ids=list(range(8)))
    global LAST_RESULT
    LAST_RESULT = res

    out = np.zeros((B, N, D), np.float32)
    for b in range(B):
        tot = res.results[2 * b]["out"] + res.results[2 * b + 1]["out"]
        # undo n' = (u, n) row order -> n2 = 2n + u
        out[b] = tot.reshape(2, 512, D).transpose(1, 0, 2).reshape(N, D) + bo_
    return out



# revision 7
# speedup vs baseline: 1.3050x; 1.3050x over previous
"""Differential multi-headed attention on 8 Trainium2 NeuronCores.

Sharding: core c = (batch b = c // 2, head-group g = c % 2).  Each core
computes 4 of the 8 differential heads for one batch element, including
the Q/K/V projections restricted to its 512 output dims, the differential
attention, and a partial output projection.  The host sums the two
partial outputs per batch (the "all-reduce"), un-permutes rows, and adds
the output bias.

Device schedule notes (v2):
 - The TRN2 PE has a p-state ramp (0.65 -> 1.2 -> 2.4 GHz over 3us of
   *continuous* execution; any idle gap resets it).  The kernel is
   therefore organized as a software pipeline that keeps the PE stream
   gap-free: the Q/K/V projection matmuls for head j+1 are interleaved
   as filler between the attention matmuls of head j.
 - The softmax key-sum (fold tree + ones-matmul partition reduction) for
   one half is issued *after* the next half's first score chunks, so the
   PE never waits on the DVE/GPSIMD fold tree.  Reciprocals and the
   differential combine are issued a further 3 chunks later, when the
   DRAM broadcast bounce has already landed, so the DVE never
   head-of-line blocks on a DMA.
 - Engines: ACT does only the exps; DVE does biases, most folds, recips
   and the differential combine; GPSIMD does the PSUM->SBUF copies and
   two mid-chain folds.
 - PSUM budget (8 banks): score ring 2x[128,1024] (4), AV accumulator
   [128,1024] (2), projection scratch [128,512] (1), sums [1,512] (1).
"""

import math
from contextlib import ExitStack

import ml_dtypes
import numpy as np

import concourse.bass as bass
import concourse.mybir as mybir
from concourse import bacc
import concourse.tile as tile
from concourse.bass_utils import run_bass_kernel_spmd

F32 = mybir.dt.float32
BF16 = mybir.dt.bfloat16
AF = mybir.ActivationFunctionType
ALU = mybir.AluOpType

N = 1024          # sequence length
D = 1024          # model dim
HG = 512          # head-group dims per core (4 heads x 128)
NHEAD = 4         # local heads per core
SCALE = 1.0 / math.sqrt(64.0)   # 1/sqrt(dk/2)
LAMBDA_INIT = 0.8

_BUILT = None     # cached Bass module -- building + compiling is expensive
LAST_RESULT = None  # BassKernelResults from the most recent run (for test.py)


def _build():
    nc = bacc.Bacc()

    # ---- DRAM I/O (per core) ----
    xqT = nc.dram_tensor("xqT", [D, N], BF16, kind="ExternalInput")
    xkT = nc.dram_tensor("xkT", [D, N], BF16, kind="ExternalInput")
    xvT = nc.dram_tensor("xvT", [D, N], BF16, kind="ExternalInput")  # col-permuted
    wqT = nc.dram_tensor("wqT", [D, HG], BF16, kind="ExternalInput")
    wkT = nc.dram_tensor("wkT", [D, HG], BF16, kind="ExternalInput")
    wvT = nc.dram_tensor("wvT", [D, HG], BF16, kind="ExternalInput")
    woT = nc.dram_tensor("woT", [HG, D], BF16, kind="ExternalInput")
    bq = nc.dram_tensor("bq", [HG], F32, kind="ExternalInput")
    bk = nc.dram_tensor("bk", [HG], F32, kind="ExternalInput")
    bv = nc.dram_tensor("bv", [HG], F32, kind="ExternalInput")
    lamneg = nc.dram_tensor("lamneg", [1, 1], F32, kind="ExternalInput")
    out = nc.dram_tensor("out", [N, D], F32, kind="ExternalOutput")

    with tile.TileContext(nc) as tc, ExitStack() as ctx:
        const = ctx.enter_context(tc.tile_pool(name="const", bufs=1))
        ones_bf = const.tile([128, 1], BF16, name="ones_bf")
        nc.vector.memset(ones_bf[:], 1.0)
        lamneg_bc = const.tile([128, 1], F32, name="lamneg_bc")
        nc.sync.dma_start(out=lamneg_bc[:], in_=lamneg[0, :].partition_broadcast(128))
        bq_sb = const.tile([128, 4], F32, name="bq_sb")
        nc.sync.dma_start(out=bq_sb[:], in_=bq.rearrange("(t p) -> p t", p=128))
        bk_sb = const.tile([128, 4], F32, name="bk_sb")
        nc.sync.dma_start(out=bk_sb[:], in_=bk.rearrange("(t p) -> p t", p=128))
        bv_sb = const.tile([128, HG], F32, name="bv_sb")
        nc.sync.dma_start(out=bv_sb[:], in_=bv[None, :].to_broadcast([128, HG]))

        # Persistent activations / weights
        persist = ctx.enter_context(tc.tile_pool(name="persist", bufs=1))
        qt = [persist.tile([128, N], BF16, name=f"qt{t}") for t in range(4)]
        # qsw[j]: partitions 0:64 = qt[j][64:128], 64:128 = qt[j][0:64] -- lets
        # every (w, u) scores matmul read lhsT and rhs from the same base
        # partition.
        qsw = [persist.tile([128, N], BF16, name=f"qsw{t}") for t in range(4)]
        kt = [persist.tile([128, N], BF16, name=f"kt{t}") for t in range(4)]
        vv = [persist.tile([128, HG], BF16, name=f"vv{m}") for m in range(8)]
        oh = [persist.tile([128, N], BF16, name=f"oh{j}") for j in range(NHEAD)]
        wo_sb = [persist.tile([128, N], BF16, name=f"wo{t}") for t in range(4)]

        xw = ctx.enter_context(tc.tile_pool(name="xw", bufs=1))
        xs = {}
        ws = {}
        for nm, xd, wd in (("q", xqT, wqT), ("k", xkT, wkT), ("v", xvT, wvT)):
            for d in range(8):
                xst = xw.tile([128, N], BF16, name=f"x{nm}{d}")
                nc.sync.dma_start(out=xst[:], in_=xd[d * 128:(d + 1) * 128, :])
                xs[(nm, d)] = xst
                wst = xw.tile([128, HG], BF16, name=f"w{nm}{d}")
                nc.sync.dma_start(out=wst[:], in_=wd[d * 128:(d + 1) * 128, :])
                ws[(nm, d)] = wst
        for t in range(4):
            nc.sync.dma_start(out=wo_sb[t][:], in_=woT[t * 128:(t + 1) * 128, :])

        # Working pools
        etp = ctx.enter_context(tc.tile_pool(name="etp", bufs=9))
        coefp = ctx.enter_context(tc.tile_pool(name="coefp", bufs=4))
        fap = ctx.enter_context(tc.tile_pool(name="fap", bufs=3))
        ffp = ctx.enter_context(tc.tile_pool(name="ffp", bufs=2))
        bcastp = ctx.enter_context(tc.tile_pool(name="bcastp", bufs=3))
        drb = ctx.enter_context(tc.tile_pool(name="drb", bufs=4, space="DRAM"))
        osb = ctx.enter_context(tc.tile_pool(name="osb", bufs=3))
        tmpp = ctx.enter_context(tc.tile_pool(name="tmpp", bufs=2))
        ostg = ctx.enter_context(tc.tile_pool(name="ostg", bufs=2))

        # PSUM pools -- exactly 8 banks
        psp = ctx.enter_context(tc.tile_pool(name="psp", bufs=2, space="PSUM"))
        pops = ctx.enter_context(tc.tile_pool(name="pops", bufs=1, space="PSUM"))
        pscr = ctx.enter_context(tc.tile_pool(name="pscr", bufs=2, space="PSUM"))

        # ---- projection work generator (PE filler units) ----
        def proj_units(j):
            """Yield ('mm', fn) / ('post', fn) units for head j's projections."""
            for nm, dst, bias in (("q", qt[j], bq_sb), ("k", kt[j], bk_sb)):
                for half in range(2):
                    ps = pscr.tile([128, 512], F32, name=f"p{nm}{j}{half}",
                                    tag="scr")
                    for d in range(8):
                        def mm(d=d, ps=ps, nm=nm, half=half):
                            nc.tensor.matmul(
                                ps[:],
                                ws[(nm, d)][:, j * 128:(j + 1) * 128],
                                xs[(nm, d)][:, half * 512:(half + 1) * 512],
                                start=(d == 0), stop=(d == 7),
                            )
                        yield ("mm", mm)

                    def post(ps=ps, dst=dst, bias=bias, half=half):
                        nc.vector.tensor_scalar_add(
                            dst[:, half * 512:(half + 1) * 512], ps[:],
                            bias[:, j:j + 1],
                        )
                    yield ("post", post)
                if nm == "q":
                    def swap():
                        nc.sync.dma_start(out=qsw[j][0:64, :], in_=qt[j][64:128, :])
                        nc.sync.dma_start(out=qsw[j][64:128, :], in_=qt[j][0:64, :])
                    yield ("post", swap)
            # V projection in consumption order (mi, mi+4) pairs
            for mc in range(4):
                ps = pscr.tile([128, 512], F32, name=f"pv{j}{mc}", tag="scr")
                for sub, mi in ((0, mc), (1, mc + 4)):
                    for d in range(8):
                        def mm(d=d, ps=ps, sub=sub, mi=mi):
                            nc.tensor.matmul(
                                ps[:, sub * 128:(sub + 1) * 128],
                                xs[("v", d)][:, mi * 128:(mi + 1) * 128],
                                ws[("v", d)][:, j * 128:(j + 1) * 128],
                                start=(d == 0), stop=(d == 7),
                            )
                        yield ("mm", mm)

                def post(ps=ps, mc=mc):
                    for sub, mi in ((0, mc), (1, mc + 4)):
                        nc.vector.tensor_tensor(
                            out=vv[mi][:, j * 128:(j + 1) * 128],
                            in0=ps[:, sub * 128:(sub + 1) * 128],
                            in1=bv_sb[:, j * 128:(j + 1) * 128], op=ALU.add,
                        )
                yield ("post", post)

        filler = {"gen": None}

        def pull(n):
            g = filler["gen"]
            if g is None:
                return
            cnt = 0
            while cnt < n:
                u = next(g, None)
                if u is None:
                    filler["gen"] = None
                    return
                kind, fn = u
                fn()
                if kind == "mm":
                    cnt += 1

        def drain():
            g = filler["gen"]
            if g is None:
                return
            for kind, fn in g:
                fn()
            filler["gen"] = None

        # ---- head 0 projections up front ----
        filler["gen"] = proj_units(0)
        drain()

        # state carried between halves for the delayed sum reduction
        pend = {"ff": None, "bcx": None, "head": None, "half": None}
        parts = {j: {} for j in range(NHEAD)}   # per-head ot/bcx tiles
        combine_q = []

        def emit_sps(u):
            """ones-matmul partition reduction + coef copy + broadcast DMAs
            for the pending half (u-th query half).  No DVE work here."""
            ff = pend["ff"]
            hd, hf = pend["head"], pend["half"]
            sps = pscr.tile([1, 512], F32, name=f"sps{hd}{hf}{u}", tag="scr")
            nc.tensor.matmul(
                sps[:], ones_bf[:], ff[:, u * 512:(u + 1) * 512],
                start=True, stop=True,
            )
            cfs = coefp.tile([1, 512], F32, name=f"cfs{hd}{hf}{u}", tag="coef")
            nc.scalar.copy(cfs[:], sps[:])
            drs = drb.tile([1, 512], F32, name=f"drs{hd}{hf}{u}", tag="dr")
            nc.sync.dma_start(out=drs[:], in_=cfs[:])
            bcx = pend["bcx"]
            nc.sync.dma_start(
                out=bcx[:, u * 512:(u + 1) * 512],
                in_=drs[0, :].partition_broadcast(128),
            )
            if u == 1:
                parts[hd][f"bcx{hf}"] = bcx
                pend["ff"] = None
                if hf == 1:
                    combine_q.append(hd)

        def emit_combine():
            """Recips + differential combine for the oldest fully-summed
            head.  Emitted only when the broadcast DMAs have ~landed."""
            if not combine_q:
                return
            hd = combine_q.pop(0)
            st = parts[hd]
            ot0, ot1 = st["ot0"], st["ot1"]
            bcx0, bcx1 = st["bcx0"], st["bcx1"]
            nc.vector.reciprocal_approx_fast(out=bcx0[:], in_=bcx0[:])
            nc.vector.reciprocal_approx_fast(out=bcx1[:], in_=bcx1[:])
            for u in range(2):
                sl = slice(u * 512, (u + 1) * 512)
                t1u = tmpp.tile([128, 512], F32, name=f"t1u{hd}{u}", tag="t1")
                nc.vector.tensor_tensor(
                    out=t1u[:], in0=ot0[:, sl], in1=bcx0[:, sl], op=ALU.mult)
                t2u = tmpp.tile([128, 512], F32, name=f"t2u{hd}{u}", tag="t2")
                nc.vector.scalar_tensor_tensor(
                    out=t2u[:], in0=ot1[:, sl], scalar=lamneg_bc[:],
                    in1=bcx1[:, sl], op0=ALU.mult, op1=ALU.mult,
                )
                nc.vector.tensor_tensor(
                    out=oh[hd][:, sl], in0=t1u[:], in1=t2u[:], op=ALU.add)

        # ---- attention per head, pipelined with next head's projections ----
        for j in range(NHEAD):
            drain()
            filler["gen"] = proj_units(j + 1) if j + 1 < NHEAD else None
            for half, lo in ((0, 0), (1, 512)):
                ets = []
                ops = pops.tile([128, N], F32, name=f"ops{j}{half}", tag="ops")
                fa = [None] * 4
                fb = None

                def av(c, start, stop):
                    mi = (c % 2) * 4 + (c // 2)
                    for u in range(2):
                        nc.tensor.matmul(
                            ops[:, u * 512:(u + 1) * 512],
                            vv[mi][:, j * 128:(j + 1) * 128],
                            ets[c][:, u * 512:(u + 1) * 512],
                            start=start, stop=stop,
                        )

                for c in range(8):
                    mc, w = c // 2, c % 2
                    sp = psp.tile([128, N], F32, name=f"sp{j}{half}{c}", tag="sp")
                    for u in range(2):
                        qsrc = qt[j] if u == w else qsw[j]
                        nc.tensor.matmul(
                            sp[:, u * 512:(u + 1) * 512],
                            kt[j][w * 64:(w + 1) * 64,
                                  lo + mc * 128:lo + (mc + 1) * 128],
                            qsrc[w * 64:(w + 1) * 64, lo:lo + 512],
                            start=True, stop=True,
                        )
                    et = etp.tile([128, N], BF16, name=f"et{j}{half}{c}", tag="et")
                    nc.scalar.activation(et[:], sp[:], AF.Exp, scale=SCALE)
                    ets.append(et)

                    # delayed partition-sums of the previous half
                    if pend["ff"] is not None:
                        if c == 1:
                            emit_sps(0)
                        elif c == 4:
                            emit_sps(1)

                    pull(3)
                    if c >= 1:
                        av(c - 1, start=(c == 1), stop=False)
                    pull(3)

                    # fold tree: pairs (0,1) DVE, (2,3) GPSIMD, (4,5) GPSIMD,
                    # (6,7) DVE; inner combines scheduled so no engine ever
                    # head-of-line blocks on a slow producer.
                    if c == 1:
                        fa[0] = fap.tile([128, N], BF16, name=f"fa{j}{half}0",
                                         tag="fa")
                        nc.vector.tensor_tensor(out=fa[0][:], in0=ets[0][:],
                                                in1=ets[1][:], op=ALU.add)
                    elif c == 3:
                        fa[1] = fap.tile([128, N], BF16, name=f"fa{j}{half}1",
                                         tag="fa")
                        nc.gpsimd.tensor_tensor(out=fa[1][:], in0=ets[2][:],
                                                in1=ets[3][:], op=ALU.add)
                    elif c == 5:
                        fa[2] = fap.tile([128, N], BF16, name=f"fa{j}{half}2",
                                         tag="fa")
                        nc.gpsimd.tensor_tensor(out=fa[2][:], in0=ets[4][:],
                                                in1=ets[5][:], op=ALU.add)
                        fb = fap.tile([128, N], BF16, name=f"fb{j}{half}",
                                      tag="fb")
                        nc.vector.tensor_tensor(out=fb[:], in0=fa[0][:],
                                                in1=fa[1][:], op=ALU.add)
                    elif c == 7:
                        fa[3] = fap.tile([128, N], BF16, name=f"fa{j}{half}3",
                                         tag="fa")
                        nc.vector.tensor_tensor(out=fa[3][:], in0=ets[6][:],
                                                in1=ets[7][:], op=ALU.add)
                        fc = fap.tile([128, N], BF16, name=f"fc{j}{half}",
                                      tag="fc")
                        nc.vector.tensor_tensor(out=fc[:], in0=fa[2][:],
                                                in1=fa[3][:], op=ALU.add)
                        ff = ffp.tile([128, N], BF16, name=f"ff{j}{half}",
                                      tag="ff")
                        nc.vector.tensor_tensor(out=ff[:], in0=fb[:],
                                                in1=fc[:], op=ALU.add)
                        # combine of the head finished two halves ago; its
                        # broadcast landed around chunk 7 of this half.
                        emit_combine()

                av(7, start=False, stop=True)
                ot = osb.tile([128, N], F32, name=f"ot{j}{half}", tag="ot")
                nc.scalar.copy(ot[:], ops[:])
                parts[j][f"ot{half}"] = ot

                bcx = bcastp.tile([128, N], F32, name=f"bcx{j}{half}", tag="bc")
                pend.update(ff=ff, bcx=bcx, head=j, half=half)

        # ---- tail: last half's sums, final combine, output projection ----
        drain()

        ps_of = {}
        tags = ["sp", "sp", "ops", "scr", "scr"]
        pools = {"sp": psp, "ops": pops, "scr": pscr}

        def alloc_out_ps(idx, k):
            tg = tags[k % 5]
            ps = pools[tg].tile([128, 512], F32, name=f"po{idx}", tag=tg)
            ps_of[idx] = ps

        def outproj_mm(idx, jlist, start):
            nci, half = idx // 2, idx % 2
            ps = ps_of[idx]
            for jj in jlist:
                nc.tensor.matmul(
                    ps[:],
                    oh[jj][:, nci * 128:(nci + 1) * 128],
                    wo_sb[jj][:, half * 512:(half + 1) * 512],
                    start=(start and jj == jlist[0]), stop=(jj == 3),
                )

        def outproj_fin(idx):
            nci, half = idx // 2, idx % 2
            ps = ps_of[idx]
            stg = ostg.tile([128, 512], F32, name=f"stg{idx}", tag="og")
            nc.vector.tensor_copy(out=stg[:], in_=ps[:])
            nc.sync.dma_start(
                out=out[nci * 128:(nci + 1) * 128,
                        half * 512:(half + 1) * 512],
                in_=stg[:],
            )

        # wave A: pre-accumulate heads 0..2 while head 3's sum-bounce is in
        # flight; the head-3 sums + combine interleave into the same window.
        for k, idx in enumerate(range(5)):
            alloc_out_ps(idx, k)
            if k == 0:
                emit_sps(0)
            outproj_mm(idx, [0, 1, 2], start=True)
            if k == 1:
                emit_sps(1)
        emit_combine()
        for idx in range(5):
            outproj_mm(idx, [3], start=False)
            outproj_fin(idx)
        for k, idx in enumerate(range(5, 16)):
            alloc_out_ps(idx, 5 + k)
            outproj_mm(idx, [0, 1, 2, 3], start=True)
            outproj_fin(idx)

    if not nc.is_finalized():
        nc.finalize()
    return nc


def _get_built():
    global _BUILT
    if _BUILT is None:
        _BUILT = _build()
    return _BUILT


def kernel(**inputs):
    inp = {k: np.asarray(v) for k, v in inputs.items()}
    q_, k_, v_ = inp["query"], inp["key"], inp["value"]
    Wq, Wk, Wv, Wo = inp["Wq"], inp["Wk"], inp["Wv"], inp["Wo"]
    bq_, bk_, bv_, bo_ = inp["bq"], inp["bk"], inp["bv"], inp["bo"]
    B = q_.shape[0]

    lam = (np.exp(np.sum(inp["lambda_q1"].astype(np.float64) * inp["lambda_k1"].astype(np.float64)))
           - np.exp(np.sum(inp["lambda_q2"].astype(np.float64) * inp["lambda_k2"].astype(np.float64)))
           + LAMBDA_INIT)

    # value-row permutation: xv'[w*512 + m] = xv[2m + w]
    permv = np.arange(N).reshape(512, 2).T.reshape(-1)  # index i'=w*512+m -> 2m+w

    in_maps = []
    for c in range(8):
        b, g = c // 2, c % 2
        sl = slice(g * HG, (g + 1) * HG)
        bf = ml_dtypes.bfloat16
        in_maps.append({
            "xqT": np.ascontiguousarray(q_[b].T).astype(bf),
            "xkT": np.ascontiguousarray(k_[b].T).astype(bf),
            "xvT": np.ascontiguousarray(v_[b][permv].T).astype(bf),
            "wqT": np.ascontiguousarray(Wq[sl, :].T).astype(bf),
            "wkT": np.ascontiguousarray(Wk[sl, :].T).astype(bf),
            "wvT": np.ascontiguousarray(Wv[sl, :].T).astype(bf),
            "woT": np.ascontiguousarray(Wo[:, sl].T).astype(bf),
            "bq": np.ascontiguousarray(bq_[sl]),
            "bk": np.ascontiguousarray(bk_[sl]),
            "bv": np.ascontiguousarray(bv_[sl]),
            "lamneg": np.array([[-lam]], dtype=np.float32),
        })

    nc = _get_built()
    res = run_bass_kernel_spmd(nc, in_maps, core_ids=list(range(8)))
    global LAST_RESULT
    LAST_RESULT = res

    out = np.zeros((B, N, D), np.float32)
    for b in range(B):
        tot = res.results[2 * b]["out"] + res.results[2 * b + 1]["out"]
        # undo n' = (u, n) row order -> n2 = 2n + u
        out[b] = tot.reshape(2, 512, D).transpose(1, 0, 2).reshape(N, D) + bo_
    return out
